# revision 20
# baseline (speedup 1.0000x reference)
"""Trainium2 Bass kernel for a 2-layer bidirectional LSTM + Dense(1) head.

Problem shapes: x [64, 1024, 64]; per layer/direction W [Fin, 1024], U [256, 1024],
b [1024]; head Wd [512, 1], bd [1]. Output [64, 1024, 1] fp32.

Sharding: 8 cores = 2 directions x 4 batch groups (16 rows per core). Each core
runs one scan per layer over its group. The fwd/bwd halves needed by layer 1 are
exchanged between core pairs with an AllGather; everything else is core-local.

Device-side math uses the all-sigmoid LSTM formulation:
    s = sigmoid(z_hat)                  (z_hat has tanh args pre-scaled by 2)
    c = s_f * c + 2*(s_g - 0.5)*s_i     (= f*c + i*tanh(z_g))
    h_stored = (sigmoid(2c) - 0.5)*s_o  (= o*tanh(c)/2, the /2 is folded into
                                         every weight that consumes h)
Weights/x/xp/h are fp16 on device (fp32 accumulation in PSUM); c and sigmoid
evaluations are fp32.
"""
import numpy as np
from contextlib import ExitStack

import concourse.bass as bass
import concourse.bacc as bacc
import concourse.mybir as mybir
import concourse.tile as tile
from concourse.bass import ds
from concourse.bass_utils import run_bass_kernel_spmd
from concourse.masks import make_identity

F16, F32 = mybir.dt.float16, mybir.dt.float32
NP16, NP32 = np.float16, np.float32

B, T_FULL, F_IN, H = 64, 1024, 64, 256
G4 = 4 * H          # 1024 gate columns
NCORES = 8
BL = 16             # batch rows per core
CH = 8              # scan steps per xp stream chunk
AF = mybir.ActivationFunctionType
ALU = mybir.AluOpType


# ---------------------------------------------------------------- host prep --

def _gate_perm():
    # reference gate col order [i, f, g, o] -> ours [g, i, f, o]
    return np.concatenate(
        [np.arange(2 * H, 3 * H), np.arange(0, H), np.arange(H, 2 * H),
         np.arange(3 * H, 4 * H)])


def _colscale():
    s = np.ones(G4, NP32)
    s[:H] = 2.0      # tanh trick: sigma(2*z_g); g block is first
    return s


def _prep_layer(Wref, Uref, bref, h_scaled_input):
    p, cs = _gate_perm(), _colscale()
    W = Wref[:, p] * cs[None, :]
    if h_scaled_input:
        W = W * 2.0
    U = Uref[:, p] * cs[None, :] * 2.0
    b = bref[p] * cs
    return W.astype(NP32), U.astype(NP32), b.astype(NP32)


def _tile_u(U):
    # [256, 1024] -> [128, 2048], col = (k*8+m)*128 + j
    return np.ascontiguousarray(
        U.reshape(2, 128, 8, 128).transpose(1, 0, 2, 3).reshape(128, 2048))


def _tile_w1(W):
    # [512, 1024] -> [128, 4096], col = (k*8+m)*128 + j
    return np.ascontiguousarray(
        W.reshape(4, 128, 8, 128).transpose(1, 0, 2, 3).reshape(128, 4096))


def host_prep(inputs, T=T_FULL, ncores=NCORES):
    """Returns list of per-core input dicts for run_bass_kernel_spmd."""
    x = np.asarray(inputs['x'])[:, :T, :]
    Wd = np.asarray(inputs['Wd'])

    Wh = {}
    for d, sfx in (('f', 'f'), ('b', 'b')):
        W0, U0, b0 = _prep_layer(np.asarray(inputs['W0' + sfx]),
                                 np.asarray(inputs['U0' + sfx]),
                                 np.asarray(inputs['b0' + sfx]), False)
        W1, U1, b1 = _prep_layer(np.asarray(inputs['W1' + sfx]),
                                 np.asarray(inputs['U1' + sfx]),
                                 np.asarray(inputs['b1' + sfx]), True)
        Wh[d] = (W0, U0, b0, W1, U1, b1)

    in_maps = []
    for c in range(ncores):
        d = 'f' if c % 2 == 0 else 'b'
        g = c // 2
        W0, U0, b0, W1, U1, b1 = Wh[d]

        xg = np.transpose(x[g * BL:(g + 1) * BL], (1, 0, 2))  # [T, BL, F]
        if d == 'b':
            xg = xg[::-1]
        # xT [65, 16T]: rows = feature, col = t*16+b; last row = ones
        xT = np.transpose(xg, (2, 0, 1)).reshape(F_IN, T * BL)
        xT = np.concatenate([xT, np.ones((1, T * BL), NP32)], 0).astype(NP16)

        # w0 aug with bias row -> [65, 1024]
        w0 = np.concatenate([W0, b0[None, :]], 0).astype(NP16)

        W1h = W1
        if d == 'b':
            W1h = np.concatenate([W1[H:2 * H], W1[:H]], 0)  # own-dir rows first
        wdh = (2.0 * Wd[:H, 0]) if d == 'f' else (2.0 * Wd[H:, 0])

        sel0 = 1.0 if c % 2 == 1 else 0.0   # peer slot: even core's peer is slot1
        in_maps.append({
            'xT': np.ascontiguousarray(xT),
            'u0': _tile_u(U0).astype(NP16),
            'w0': np.ascontiguousarray(w0),
            'u1': _tile_u(U1).astype(NP16),
            'w1': _tile_w1(W1h).astype(NP16),
            'b1': np.ascontiguousarray(b1.reshape(8, 128).T).astype(NP32),
            'wd': np.ascontiguousarray(wdh.reshape(2, 128).T).astype(NP16),
            'sel0': np.full((128, 1), sel0, NP32),
            'sel1': np.full((128, 1), 1.0 - sel0, NP32),
        })
    return in_maps


def host_post(results, inputs, T=T_FULL):
    bd = np.asarray(inputs['bd'])
    y = np.zeros((B, T, 1), NP32)
    for g in range(NCORES // 2):
        yf = results[2 * g]['y'].reshape(T, BL)
        yb = results[2 * g + 1]['y'].reshape(T, BL)[::-1]
        y[g * BL:(g + 1) * BL, :, 0] = (yf + yb).T + bd[0]
    return y


# ------------------------------------------------------------- device build --

def _scan_state(nc, pools):
    """Allocate scan state tiles (shared across sub-loops and layers)."""
    zp, sp, wp, cst, xpp = pools
    st = {}
    st['cA'] = cst.tile([128, 32], F32, tag="cA", name="cA")
    st['cB'] = cst.tile([128, 32], F32, tag="cB", name="cB")
    st['stgA'] = cst.tile([128, CH * 32], F16, tag="stgA", name="stgA")
    st['stgB'] = cst.tile([128, CH * 32], F16, tag="stgB", name="stgB")
    st['xpA'] = xpp.tile([128, CH, 128], F16, tag="xpA", name="xpA")
    st['xpB'] = xpp.tile([128, CH, 128], F16, tag="xpB", name="xpB")
    return st


def _scan_range(tc, nc, st, lo, hi, u_sb, xp_d, hst_d, ident, pools, prologue, col0=0):
    """Emit scan steps for chunk range [lo, hi) (each chunk = CH steps).

    All compute APs are static; h history lives in two chunk staging tiles
    (stgA/stgB) and is appended to the DRAM store hst_d via DMA (dynamic
    offsets are DRAM-side only). hst_d col block s = h(s) at [32s, 32s+32).
    """
    zp, sp, wp, cst, xpp = pools
    cA, cB = st['cA'], st['cB']
    stgA, stgB, xpA, xpB = st['stgA'], st['stgB'], st['xpA'], st['xpB']
    if prologue:
        nc.vector.memset(cA[:], 0.0)
        nc.vector.memset(stgB[:, (CH - 1) * 32:], 0.0)   # h(-1) = 0
        nc.sync.dma_start(xpA[:], xp_d[:, ds(0, CH * 128)])

    def step(j, xp_tile, stg, stg_prev, c_in, c_out):
        h_prev = (stg_prev[:, (CH - 1) * 32:] if j == 0
                  else stg[:, (j - 1) * 32:j * 32])
        zt = zp.tile([128, 128], F32, tag="z")
        nc.tensor.matmul(zt[:], ident[:], xp_tile[:, j, :],
                         start=True, stop=False, skip_group_check=True)
        for k in range(2):
            for m in range(8):
                nc.tensor.matmul(
                    zt[:, ds(m * 16, 16)],
                    u_sb[:, ds((k * 8 + m) * 128, 128)],
                    h_prev[:, ds(k * 16, 16)],
                    start=False, stop=(m == 7 and k == 1), skip_group_check=True)
        # gate col order [g, i, f, o]; cell state kept as c' = c/2 so the
        # update is a plain add: c' = (s_g-.5)*s_i + s_f*c'
        s_t = sp.tile([128, 128], F32, tag="s")
        nc.scalar.activation(s_t[:, 0:64], zt[:, 0:64], AF.Sigmoid)
        nc.scalar.activation(s_t[:, 64:128], zt[:, 64:128], AF.Sigmoid)
        t1 = wp.tile([128, 32], F32, tag="t1")
        nc.vector.scalar_tensor_tensor(t1[:], s_t[:, 0:32], -0.5, s_t[:, 32:64],
                                       op0=ALU.add, op1=ALU.mult)
        tm = wp.tile([128, 32], F32, tag="tm")
        nc.vector.tensor_mul(tm[:], s_t[:, 64:96], c_in[:])
        nc.vector.tensor_add(c_out[:], t1[:], tm[:])
        sc = wp.tile([128, 32], F32, tag="sc")
        nc.scalar.activation(sc[:], c_out[:], AF.Sigmoid, scale=4.0)
        nc.vector.scalar_tensor_tensor(
            stg[:, j * 32:(j + 1) * 32],
            sc[:], -0.5, s_t[:, 96:128], op0=ALU.add, op1=ALU.mult)

    def chunk(xp_tile, stg, stg_prev):
        for j in range(CH):
            even = (j % 2 == 0)
            step(j, xp_tile, stg, stg_prev, cA if even else cB, cB if even else cA)

    with tc.For_i(lo, hi, 2, staggered_reset=True,
                  hint_engines=(mybir.EngineType.PE, mybir.EngineType.DVE,
                                mybir.EngineType.Activation)) as i:
        nc.sync.dma_start(xpB[:], xp_d[:, ds(i * (CH * 128) + CH * 128, CH * 128)])
        chunk(xpA, stgA, stgB)
        nc.sync.dma_start(hst_d[:, ds(i * (CH * 32) - col0, CH * 32)], stgA[:])
        nc.sync.dma_start(xpA[:], xp_d[:, ds(i * (CH * 128) + 2 * CH * 128, CH * 128)])
        chunk(xpB, stgB, stgA)
        nc.sync.dma_start(hst_d[:, ds(i * (CH * 32) + CH * 32 - col0, CH * 32)], stgB[:])


def build_nc(T=T_FULL, pad_init=False, finalize=True):
    nc = bacc.Bacc(None, num_devices=NCORES)
    NT = BL * T

    xT = nc.declare_dram_parameter("xT", [F_IN + 1, NT], F16, isOutput=False)
    u0 = nc.declare_dram_parameter("u0", [128, 2048], F16, isOutput=False)
    w0 = nc.declare_dram_parameter("w0", [F_IN + 1, 1024], F16, isOutput=False)
    u1 = nc.declare_dram_parameter("u1", [128, 2048], F16, isOutput=False)
    w1 = nc.declare_dram_parameter("w1", [128, 4096], F16, isOutput=False)
    b1 = nc.declare_dram_parameter("b1", [128, 8], F32, isOutput=False)
    wd = nc.declare_dram_parameter("wd", [128, 2], F16, isOutput=False)
    sel0 = nc.declare_dram_parameter("sel0", [128, 1], F32, isOutput=False)
    sel1 = nc.declare_dram_parameter("sel1", [128, 1], F32, isOutput=False)
    y = nc.declare_dram_parameter("y", [1, NT], F32, isOutput=True)

    xp0_d = nc.dram_tensor("xp0_d", [128, (T + 2 * CH) * 128], F16)
    xp1_d = nc.dram_tensor("xp1_d", [128, (T + 2 * CH) * 128], F16)
    NQ = 4 if (T % 128 == 0 and (T // CH // 4) % 2 == 0) else 1
    qcols = 32 * T // NQ
    exch_in = nc.dram_tensor("exch_in", [NQ, 128, qcols], F16)
    own1_d = nc.dram_tensor("own1_d", [128, 32 * T], F16)
    # note: addr_space="Shared" is rejected for 2-core groups; Local works.
    exch_out = nc.dram_tensor("exch_out", [NQ, 2, 128, qcols], F16)

    with tile.TileContext(nc) as tc, ExitStack() as ctx:
        const = ctx.enter_context(tc.tile_pool(name="const", bufs=1))
        xpp = ctx.enter_context(tc.tile_pool(name="xpp", bufs=1))
        gst = ctx.enter_context(tc.tile_pool(name="gst", bufs=3))
        peerp = ctx.enter_context(tc.tile_pool(name="peerp", bufs=2))
        sp = ctx.enter_context(tc.tile_pool(name="sp", bufs=3))
        wp = ctx.enter_context(tc.tile_pool(name="wp", bufs=3))
        cst = ctx.enter_context(tc.tile_pool(name="cst", bufs=1))
        zp = ctx.enter_context(tc.tile_pool(name="zp", bufs=2, space="PSUM"))
        gps = ctx.enter_context(tc.tile_pool(name="gps", bufs=4, space="PSUM"))
        hps = ctx.enter_context(tc.tile_pool(name="hps", bufs=2, space="PSUM"))

        # --- load parameters to SBUF
        u0_sb = const.tile([128, 2048], F16, tag="u0")
        nc.sync.dma_start(u0_sb[:], u0[:])
        u1_sb = const.tile([128, 2048], F16, tag="u1")
        nc.sync.dma_start(u1_sb[:], u1[:])
        w0_sb = const.tile([F_IN + 1, 1024], F16, tag="w0")
        nc.sync.dma_start(w0_sb[:], w0[:])
        w1_sb = const.tile([128, 4096], F16, tag="w1")
        nc.sync.dma_start(w1_sb[:], w1[:])
        b1_sb = const.tile([128, 8], F32, tag="b1")
        nc.sync.dma_start(b1_sb[:], b1[:])
        wd_sb = const.tile([128, 2], F16, tag="wd")
        nc.sync.dma_start(wd_sb[:], wd[:])
        sel0_sb = const.tile([128, 1], F32, tag="sel0")
        nc.sync.dma_start(sel0_sb[:], sel0[:])
        sel1_sb = const.tile([128, 1], F32, tag="sel1")
        nc.sync.dma_start(sel1_sb[:], sel1[:])
        xT_sb = const.tile([F_IN + 1, NT], F16, tag="xT")
        nc.sync.dma_start(xT_sb[:], xT[:])
        ident = const.tile([128, 128], F16, tag="ident")
        make_identity(nc, ident[:])
        warm = const.tile([128, 1], F32, tag="warm")
        nc.vector.memset(warm[:], 0.0)
        nc.scalar.activation(warm[:], warm[:], AF.Sigmoid)
        if pad_init:
            # Only to satisfy the simulator's NaN-canary on the prefetch
            # overrun region; the values are never consumed by compute.
            zpad = const.tile([128, 2 * CH * 128], F16, tag="zpad")
            nc.vector.memset(zpad[:], 0.0)
            nc.sync.dma_start(xp0_d[:, ds(T * 128, 2 * CH * 128)], zpad[:])
            nc.sync.dma_start(xp1_d[:, ds(T * 128, 2 * CH * 128)], zpad[:])

        scan_pools = (zp, sp, wp, cst, xpp)

        # --- xp0 = [x;1] @ [W0;b0]  -> xp0_d (t-blocked layout)
        for tci in range(T // 32):
            asm = gst.tile([128, 32, 128], F16, tag="asm")
            for m in range(8):
                ps = gps.tile([128, 512], F32, tag="gps")
                nc.tensor.matmul(ps[:], w0_sb[:, ds(m * 128, 128)],
                                 xT_sb[:, ds(tci * 512, 512)],
                                 start=True, stop=True)
                if m % 2 == 0:
                    nc.vector.tensor_copy(asm[:, :, ds(m * 16, 16)],
                                          ps.rearrange("p (t b) -> p t b", b=16))
                else:
                    nc.scalar.copy(asm[:, :, ds(m * 16, 16)],
                                   ps.rearrange("p (t b) -> p t b", b=16))
            nc.sync.dma_start(xp0_d[:, ds(tci * 4096, 4096)], asm[:])

        tc.strict_bb_all_engine_barrier()
        # --- L0 scan in quarters; exchange each quarter as it completes
        rgroups = [[2 * g, 2 * g + 1] for g in range(NCORES // 2)]
        st = _scan_state(nc, scan_pools)
        qchunks = T // CH // NQ
        ccs = []
        for q in range(NQ):
            _scan_range(tc, nc, st, q * qchunks, (q + 1) * qchunks,
                        u0_sb, xp0_d, exch_in[q], ident, scan_pools,
                        prologue=(q == 0), col0=q * qcols)
            cc = nc.gpsimd.collective_compute(
                "AllGather", ALU.bypass, replica_groups=rgroups,
                ins=[exch_in[q]], outs=[exch_out[q]])
            ccs.append(cc)

        # --- xp1 = [own; peer_reversed] @ W1 + b1 -> xp1_d
        # tci descending so the first-needed peer quarters are the
        # first-exchanged ones; peer DMAs gate on their quarter's collective.
        for tci in range(T // 32 - 1, -1, -1):
            t0 = tci * 32
            q = (32 * (T - 32 - t0)) // qcols
            qoff = 32 * (T - 32 - t0) - q * qcols
            q_own = (32 * t0) // qcols
            ownc = peerp.tile([128, 32, 32], F16, tag="ownc")
            do = nc.sync.dma_start(
                ownc[:], exch_in[q_own, :, ds(32 * t0 - q_own * qcols, 1024)]
                .rearrange("p (t c) -> p t c", c=32))
            tile.add_dep_helper(do.ins, ccs[q_own].ins, reason="own q written")
            s0c = peerp.tile([128, 32, 32], F16, tag="s0c")
            d0 = nc.sync.dma_start(s0c[:],
                                   exch_out[q, 0, :, ds(qoff, 1024)]
                                   .rearrange("p (t c) -> p t c", c=32))
            s1c = peerp.tile([128, 32, 32], F16, tag="s1c")
            d1 = nc.sync.dma_start(s1c[:],
                                   exch_out[q, 1, :, ds(qoff, 1024)]
                                   .rearrange("p (t c) -> p t c", c=32))
            tile.add_dep_helper(d0.ins, ccs[q].ins, reason="peer q ready")
            tile.add_dep_helper(d1.ins, ccs[q].ins, reason="peer q ready")
            pm = peerp.tile([128, 32, 32], F16, tag="pm")
            nc.vector.tensor_scalar(pm[:], s1c[:, ::-1, :], sel1_sb[:, 0:1], None,
                                    op0=ALU.mult)
            pc = peerp.tile([128, 32, 32], F16, tag="pc")
            nc.vector.scalar_tensor_tensor(pc[:], s0c[:, ::-1, :], sel0_sb[:, 0:1],
                                           pm[:], op0=ALU.mult, op1=ALU.add)
            asm = gst.tile([128, 32, 128], F16, tag="asm")
            for m in range(8):
                ps = gps.tile([128, 512], F32, tag="gps")
                for k in range(4):
                    rhs = (ownc[:, :, ds(k * 16, 16)] if k < 2
                           else pc[:, :, ds((k - 2) * 16, 16)])
                    nc.tensor.matmul(ps[:], w1_sb[:, ds((k * 8 + m) * 128, 128)],
                                     rhs, start=(k == 0), stop=(k == 3))
                nc.vector.tensor_scalar(asm[:, :, ds(m * 16, 16)],
                                         ps.rearrange("p (t b) -> p t b", b=16),
                                         b1_sb[:, ds(m, 1)], None, op0=ALU.add)
            nc.sync.dma_start(xp1_d[:, ds(tci * 4096, 4096)], asm[:])

        tc.strict_bb_all_engine_barrier()
        # --- L1 scan
        _scan_range(tc, nc, st, 0, T // CH, u1_sb, xp1_d, own1_d, ident,
                    scan_pools, prologue=True)
        tc.strict_bb_all_engine_barrier()

        # --- head: y = h1_own @ wd_half (partial; host sums pairs)
        y_sb = const.tile([1, NT], F32, tag="ysb")
        for tci in range(T // 32):
            t0 = tci * 32
            own1c = peerp.tile([128, 32, 32], F16, tag="own1c")
            nc.sync.dma_start(own1c[:], own1_d[:, ds(32 * t0, 1024)]
                              .rearrange("p (t c) -> p t c", c=32))
            ps = hps.tile([1, 512], F32, tag="hps")
            for k in range(2):
                nc.tensor.matmul(ps[:], wd_sb[:, ds(k, 1)],
                                 own1c[:, :, ds(k * 16, 16)],
                                 start=(k == 0), stop=(k == 1))
            nc.vector.tensor_copy(y_sb[:, ds(tci * 512, 512)], ps[:])
        nc.sync.dma_start(y[:], y_sb[:])

    if finalize:
        nc.finalize()
    return nc


_NC_CACHE = {}


def _get_nc(T):
    if T not in _NC_CACHE:
        _NC_CACHE[T] = build_nc(T)
    return _NC_CACHE[T]


def kernel(**inputs) -> np.ndarray:
    T = np.asarray(inputs['x']).shape[1]
    in_maps = host_prep(inputs, T=T)
    nc = _get_nc(T)
    res = run_bass_kernel_spmd(nc, in_maps, core_ids=list(range(NCORES)))
    return host_post(res.results, inputs, T=T)


if __name__ == "__main__":
    rng = np.random.default_rng(0)
    Tt = 64
    demo = {
        'x': rng.standard_normal((B, Tt, F_IN), NP32),
        'Wd': rng.standard_normal((2 * H, 1), NP32) * 0.04,
        'bd': np.zeros(1, NP32),
    }
    for sfx in ('f', 'b'):
        demo[f'W0{sfx}'] = rng.standard_normal((F_IN, G4), NP32) * 0.125
        demo[f'U0{sfx}'] = rng.standard_normal((H, G4), NP32) * 0.0625
        demo[f'b0{sfx}'] = np.zeros(G4, NP32)
        demo[f'W1{sfx}'] = rng.standard_normal((2 * H, G4), NP32) * 0.044
        demo[f'U1{sfx}'] = rng.standard_normal((H, G4), NP32) * 0.0625
        demo[f'b1{sfx}'] = np.zeros(G4, NP32)
    out = kernel(**demo)
    print("kernel output:", out.shape, out.dtype, np.abs(out).max())


# revision 21
# speedup vs baseline: 1.2410x; 1.2410x over previous
"""Trainium2 Bass kernel for a 2-layer bidirectional LSTM + Dense(1) head.

Problem shapes: x [64, 1024, 64]; per layer/direction W [Fin, 1024], U [256, 1024],
b [1024]; head Wd [512, 1], bd [1]. Output [64, 1024, 1] fp32.

Sharding: 8 cores = 2 directions x 4 batch groups (16 rows per core). Each core
runs one scan per layer over its group. The fwd/bwd halves needed by layer 1 are
exchanged between core pairs with an AllGather; everything else is core-local.

Device-side math uses the all-sigmoid LSTM formulation:
    s = sigmoid(z_hat)                  (z_hat has tanh args pre-scaled by 2)
    c = s_f * c + 2*(s_g - 0.5)*s_i     (= f*c + i*tanh(z_g))
    h_stored = (sigmoid(2c) - 0.5)*s_o  (= o*tanh(c)/2, the /2 is folded into
                                         every weight that consumes h)
Weights/x/xp/h are fp16 on device (fp32 accumulation in PSUM); c and sigmoid
evaluations are fp32.
"""
import numpy as np
from contextlib import ExitStack

import concourse.bass as bass
import concourse.bacc as bacc
import concourse.mybir as mybir
import concourse.tile as tile
from concourse.bass import ds
from concourse.bass_utils import run_bass_kernel_spmd
from concourse.masks import make_identity

F16, F32 = mybir.dt.float16, mybir.dt.float32
NP16, NP32 = np.float16, np.float32

B, T_FULL, F_IN, H = 64, 1024, 64, 256
G4 = 4 * H          # 1024 gate columns
NCORES = 8
BL = 16             # batch rows per core
CH = 8              # scan steps per xp stream chunk
AF = mybir.ActivationFunctionType
ALU = mybir.AluOpType


# ---------------------------------------------------------------- host prep --

def _gate_perm():
    # reference gate col order [i, f, g, o] -> ours [g, i, f, o]
    return np.concatenate(
        [np.arange(2 * H, 3 * H), np.arange(0, H), np.arange(H, 2 * H),
         np.arange(3 * H, 4 * H)])


def _colscale():
    s = np.ones(G4, NP32)
    s[:H] = 2.0      # tanh trick: sigma(2*z_g); g block is first
    return s


def _prep_layer(Wref, Uref, bref, h_scaled_input):
    p, cs = _gate_perm(), _colscale()
    W = Wref[:, p] * cs[None, :]
    if h_scaled_input:
        W = W * 2.0
    U = Uref[:, p] * cs[None, :] * 2.0
    b = bref[p] * cs
    return W.astype(NP32), U.astype(NP32), b.astype(NP32)


def _tile_u(U):
    # [256, 1024] -> [128, 2048], col = (k*8+m)*128 + j
    return np.ascontiguousarray(
        U.reshape(2, 128, 8, 128).transpose(1, 0, 2, 3).reshape(128, 2048))


def _tile_w1(W):
    # [512, 1024] -> [128, 4096], col = (k*8+m)*128 + j
    return np.ascontiguousarray(
        W.reshape(4, 128, 8, 128).transpose(1, 0, 2, 3).reshape(128, 4096))


def host_prep(inputs, T=T_FULL, ncores=NCORES):
    """Returns list of per-core input dicts for run_bass_kernel_spmd."""
    x = np.asarray(inputs['x'])[:, :T, :]
    Wd = np.asarray(inputs['Wd'])

    Wh = {}
    for d, sfx in (('f', 'f'), ('b', 'b')):
        W0, U0, b0 = _prep_layer(np.asarray(inputs['W0' + sfx]),
                                 np.asarray(inputs['U0' + sfx]),
                                 np.asarray(inputs['b0' + sfx]), False)
        W1, U1, b1 = _prep_layer(np.asarray(inputs['W1' + sfx]),
                                 np.asarray(inputs['U1' + sfx]),
                                 np.asarray(inputs['b1' + sfx]), True)
        Wh[d] = (W0, U0, b0, W1, U1, b1)

    in_maps = []
    for c in range(ncores):
        d = 'f' if c % 2 == 0 else 'b'
        g = c // 2
        W0, U0, b0, W1, U1, b1 = Wh[d]

        xg = np.transpose(x[g * BL:(g + 1) * BL], (1, 0, 2))  # [T, BL, F]
        if d == 'b':
            xg = xg[::-1]
        # xT [65, 16T]: rows = feature, col = t*16+b; last row = ones
        xT = np.transpose(xg, (2, 0, 1)).reshape(F_IN, T * BL)
        xT = np.concatenate([xT, np.ones((1, T * BL), NP32)], 0).astype(NP16)

        # w0 aug with bias row -> [65, 1024]
        w0 = np.concatenate([W0, b0[None, :]], 0).astype(NP16)

        W1h = W1
        if d == 'b':
            W1h = np.concatenate([W1[H:2 * H], W1[:H]], 0)  # own-dir rows first
        wdh = (2.0 * Wd[:H, 0]) if d == 'f' else (2.0 * Wd[H:, 0])

        sel0 = 1.0 if c % 2 == 1 else 0.0   # peer slot: even core's peer is slot1
        in_maps.append({
            'xT': np.ascontiguousarray(xT),
            'u0': _tile_u(U0).astype(NP16),
            'w0': np.ascontiguousarray(w0),
            'u1': _tile_u(U1).astype(NP16),
            'w1': _tile_w1(W1h).astype(NP16),
            'b1': np.ascontiguousarray(b1.reshape(8, 128).T).astype(NP32),
            'wd': np.ascontiguousarray(wdh.reshape(2, 128).T).astype(NP16),
            'sel0': np.full((128, 1), sel0, NP32),
            'sel1': np.full((128, 1), 1.0 - sel0, NP32),
        })
    return in_maps


def host_post(results, inputs, T=T_FULL):
    bd = np.asarray(inputs['bd'])
    y = np.zeros((B, T, 1), NP32)
    for g in range(NCORES // 2):
        yf = results[2 * g]['y'].reshape(T, BL)
        yb = results[2 * g + 1]['y'].reshape(T, BL)[::-1]
        y[g * BL:(g + 1) * BL, :, 0] = (yf + yb).T + bd[0]
    return y


# ------------------------------------------------------------- device build --

def _scan_state(nc, pools):
    """Allocate scan state tiles (shared across sub-loops and layers)."""
    zp, sp, wp, cst, xpp = pools
    st = {}
    st['cA'] = cst.tile([128, 32], F32, tag="cA", name="cA")
    st['cB'] = cst.tile([128, 32], F32, tag="cB", name="cB")
    st['stgA'] = cst.tile([128, CH * 32], F16, tag="stgA", name="stgA")
    st['stgB'] = cst.tile([128, CH * 32], F16, tag="stgB", name="stgB")
    st['xpA'] = xpp.tile([128, CH, 128], F16, tag="xpA", name="xpA")
    st['xpB'] = xpp.tile([128, CH, 128], F16, tag="xpB", name="xpB")
    return st


def _scan_range(tc, nc, st, lo, hi, u_sb, xp_d, hst_d, ident, pools, prologue, col0=0):
    """Emit scan steps for chunk range [lo, hi) (each chunk = CH steps).

    All compute APs are static; h history lives in two chunk staging tiles
    (stgA/stgB) and is appended to the DRAM store hst_d via DMA (dynamic
    offsets are DRAM-side only). hst_d col block s = h(s) at [32s, 32s+32).
    """
    zp, sp, wp, cst, xpp = pools
    cA, cB = st['cA'], st['cB']
    stgA, stgB, xpA, xpB = st['stgA'], st['stgB'], st['xpA'], st['xpB']
    if prologue:
        nc.vector.memset(cA[:], 0.0)
        nc.vector.memset(stgB[:, (CH - 1) * 32:], 0.0)   # h(-1) = 0
        nc.sync.dma_start(xpA[:], xp_d[:, ds(0, CH * 128)])

    def step(j, xp_tile, stg, stg_prev, c_in, c_out):
        h_prev = (stg_prev[:, (CH - 1) * 32:] if j == 0
                  else stg[:, (j - 1) * 32:j * 32])
        zt = zp.tile([128, 128], F32, tag="z")
        nc.tensor.matmul(zt[:], ident[:], xp_tile[:, j, :],
                         start=True, stop=False, skip_group_check=True)
        for k in range(2):
            for m in range(8):
                nc.tensor.matmul(
                    zt[:, ds(m * 16, 16)],
                    u_sb[:, ds((k * 8 + m) * 128, 128)],
                    h_prev[:, ds(k * 16, 16)],
                    start=False, stop=(m == 7 and k == 1), skip_group_check=True)
        # gate col order [g, i, f, o]; cell state kept as c' = c/2 so the
        # update is a plain add: c' = (s_g-.5)*s_i + s_f*c'
        s_t = sp.tile([128, 128], F32, tag="s")
        nc.scalar.activation(s_t[:], zt[:], AF.Sigmoid)
        t1 = wp.tile([128, 32], F32, tag="t1")
        nc.vector.scalar_tensor_tensor(t1[:], s_t[:, 0:32], -0.5, s_t[:, 32:64],
                                       op0=ALU.add, op1=ALU.mult)
        tm = wp.tile([128, 32], F32, tag="tm")
        nc.vector.tensor_mul(tm[:], s_t[:, 64:96], c_in[:])
        nc.vector.tensor_add(c_out[:], t1[:], tm[:])
        sc = wp.tile([128, 32], F32, tag="sc")
        nc.scalar.activation(sc[:], c_out[:], AF.Sigmoid, scale=4.0)
        nc.vector.scalar_tensor_tensor(
            stg[:, j * 32:(j + 1) * 32],
            sc[:], -0.5, s_t[:, 96:128], op0=ALU.add, op1=ALU.mult)

    def chunk(xp_tile, stg, stg_prev):
        for j in range(CH):
            even = (j % 2 == 0)
            step(j, xp_tile, stg, stg_prev, cA if even else cB, cB if even else cA)

    with tc.For_i(lo, hi, 2, staggered_reset=True,
                  hint_engines=(mybir.EngineType.PE, mybir.EngineType.DVE,
                                mybir.EngineType.Activation)) as i:
        nc.sync.dma_start(xpB[:], xp_d[:, ds(i * (CH * 128) + CH * 128, CH * 128)])
        chunk(xpA, stgA, stgB)
        nc.sync.dma_start(hst_d[:, ds(i * (CH * 32) - col0, CH * 32)], stgA[:])
        nc.sync.dma_start(xpA[:], xp_d[:, ds(i * (CH * 128) + 2 * CH * 128, CH * 128)])
        chunk(xpB, stgB, stgA)
        nc.sync.dma_start(hst_d[:, ds(i * (CH * 32) + CH * 32 - col0, CH * 32)], stgB[:])


def build_nc(T=T_FULL, pad_init=False, finalize=True):
    nc = bacc.Bacc(None, num_devices=NCORES)
    NT = BL * T

    xT = nc.declare_dram_parameter("xT", [F_IN + 1, NT], F16, isOutput=False)
    u0 = nc.declare_dram_parameter("u0", [128, 2048], F16, isOutput=False)
    w0 = nc.declare_dram_parameter("w0", [F_IN + 1, 1024], F16, isOutput=False)
    u1 = nc.declare_dram_parameter("u1", [128, 2048], F16, isOutput=False)
    w1 = nc.declare_dram_parameter("w1", [128, 4096], F16, isOutput=False)
    b1 = nc.declare_dram_parameter("b1", [128, 8], F32, isOutput=False)
    wd = nc.declare_dram_parameter("wd", [128, 2], F16, isOutput=False)
    sel0 = nc.declare_dram_parameter("sel0", [128, 1], F32, isOutput=False)
    sel1 = nc.declare_dram_parameter("sel1", [128, 1], F32, isOutput=False)
    y = nc.declare_dram_parameter("y", [1, NT], F32, isOutput=True)

    xp0_d = nc.dram_tensor("xp0_d", [128, (T + 2 * CH) * 128], F16)
    xp1_d = nc.dram_tensor("xp1_d", [128, (T + 2 * CH) * 128], F16)
    NQ = 4 if (T % 128 == 0 and (T // CH // 4) % 2 == 0) else 1
    qcols = 32 * T // NQ
    exch_in = nc.dram_tensor("exch_in", [NQ, 128, qcols], F16)
    own1_d = nc.dram_tensor("own1_d", [128, 32 * T], F16)
    # note: addr_space="Shared" is rejected for 2-core groups; Local works.
    exch_out = nc.dram_tensor("exch_out", [NQ, 2, 128, qcols], F16)

    with tile.TileContext(nc) as tc, ExitStack() as ctx:
        const = ctx.enter_context(tc.tile_pool(name="const", bufs=1))
        xpp = ctx.enter_context(tc.tile_pool(name="xpp", bufs=1))
        gst = ctx.enter_context(tc.tile_pool(name="gst", bufs=3))
        peerp = ctx.enter_context(tc.tile_pool(name="peerp", bufs=2))
        sp = ctx.enter_context(tc.tile_pool(name="sp", bufs=3))
        wp = ctx.enter_context(tc.tile_pool(name="wp", bufs=3))
        cst = ctx.enter_context(tc.tile_pool(name="cst", bufs=1))
        zp = ctx.enter_context(tc.tile_pool(name="zp", bufs=2, space="PSUM"))
        gps = ctx.enter_context(tc.tile_pool(name="gps", bufs=4, space="PSUM"))
        hps = ctx.enter_context(tc.tile_pool(name="hps", bufs=2, space="PSUM"))

        # --- load parameters to SBUF
        u0_sb = const.tile([128, 2048], F16, tag="u0")
        nc.sync.dma_start(u0_sb[:], u0[:])
        u1_sb = const.tile([128, 2048], F16, tag="u1")
        nc.sync.dma_start(u1_sb[:], u1[:])
        w0_sb = const.tile([F_IN + 1, 1024], F16, tag="w0")
        nc.sync.dma_start(w0_sb[:], w0[:])
        w1_sb = const.tile([128, 4096], F16, tag="w1")
        nc.sync.dma_start(w1_sb[:], w1[:])
        b1_sb = const.tile([128, 8], F32, tag="b1")
        nc.sync.dma_start(b1_sb[:], b1[:])
        wd_sb = const.tile([128, 2], F16, tag="wd")
        nc.sync.dma_start(wd_sb[:], wd[:])
        sel0_sb = const.tile([128, 1], F32, tag="sel0")
        nc.sync.dma_start(sel0_sb[:], sel0[:])
        sel1_sb = const.tile([128, 1], F32, tag="sel1")
        nc.sync.dma_start(sel1_sb[:], sel1[:])
        xT_sb = const.tile([F_IN + 1, NT], F16, tag="xT")
        nc.sync.dma_start(xT_sb[:], xT[:])
        ident = const.tile([128, 128], F16, tag="ident")
        make_identity(nc, ident[:])
        warm = const.tile([128, 1], F32, tag="warm")
        nc.vector.memset(warm[:], 0.0)
        nc.scalar.activation(warm[:], warm[:], AF.Sigmoid)
        if pad_init:
            # Only to satisfy the simulator's NaN-canary on the prefetch
            # overrun region; the values are never consumed by compute.
            zpad = const.tile([128, 2 * CH * 128], F16, tag="zpad")
            nc.vector.memset(zpad[:], 0.0)
            nc.sync.dma_start(xp0_d[:, ds(T * 128, 2 * CH * 128)], zpad[:])
            nc.sync.dma_start(xp1_d[:, ds(T * 128, 2 * CH * 128)], zpad[:])

        scan_pools = (zp, sp, wp, cst, xpp)

        # --- xp0 = [x;1] @ [W0;b0]  -> xp0_d (t-blocked layout)
        for tci in range(T // 32):
            asm = gst.tile([128, 32, 128], F16, tag="asm")
            for m in range(8):
                ps = gps.tile([128, 512], F32, tag="gps")
                nc.tensor.matmul(ps[:], w0_sb[:, ds(m * 128, 128)],
                                 xT_sb[:, ds(tci * 512, 512)],
                                 start=True, stop=True)
                if m % 2 == 0:
                    nc.vector.tensor_copy(asm[:, :, ds(m * 16, 16)],
                                          ps.rearrange("p (t b) -> p t b", b=16))
                else:
                    nc.scalar.copy(asm[:, :, ds(m * 16, 16)],
                                   ps.rearrange("p (t b) -> p t b", b=16))
            nc.sync.dma_start(xp0_d[:, ds(tci * 4096, 4096)], asm[:])

        tc.strict_bb_all_engine_barrier()
        # --- L0 scan in quarters; exchange each quarter as it completes
        rgroups = [[2 * g, 2 * g + 1] for g in range(NCORES // 2)]
        st = _scan_state(nc, scan_pools)
        qchunks = T // CH // NQ
        ccs = []
        for q in range(NQ):
            _scan_range(tc, nc, st, q * qchunks, (q + 1) * qchunks,
                        u0_sb, xp0_d, exch_in[q], ident, scan_pools,
                        prologue=(q == 0), col0=q * qcols)
            cc = nc.gpsimd.collective_compute(
                "AllGather", ALU.bypass, replica_groups=rgroups,
                ins=[exch_in[q]], outs=[exch_out[q]])
            ccs.append(cc)

        # --- xp1 = [own; peer_reversed] @ W1 + b1 -> xp1_d
        # tci descending so the first-needed peer quarters are the
        # first-exchanged ones; peer DMAs gate on their quarter's collective.
        for tci in range(T // 32 - 1, -1, -1):
            t0 = tci * 32
            q = (32 * (T - 32 - t0)) // qcols
            qoff = 32 * (T - 32 - t0) - q * qcols
            q_own = (32 * t0) // qcols
            ownc = peerp.tile([128, 32, 32], F16, tag="ownc")
            do = nc.sync.dma_start(
                ownc[:], exch_in[q_own, :, ds(32 * t0 - q_own * qcols, 1024)]
                .rearrange("p (t c) -> p t c", c=32))
            tile.add_dep_helper(do.ins, ccs[q_own].ins, reason="own q written")
            s0c = peerp.tile([128, 32, 32], F16, tag="s0c")
            d0 = nc.sync.dma_start(s0c[:],
                                   exch_out[q, 0, :, ds(qoff, 1024)]
                                   .rearrange("p (t c) -> p t c", c=32))
            s1c = peerp.tile([128, 32, 32], F16, tag="s1c")
            d1 = nc.sync.dma_start(s1c[:],
                                   exch_out[q, 1, :, ds(qoff, 1024)]
                                   .rearrange("p (t c) -> p t c", c=32))
            tile.add_dep_helper(d0.ins, ccs[q].ins, reason="peer q ready")
            tile.add_dep_helper(d1.ins, ccs[q].ins, reason="peer q ready")
            pm = peerp.tile([128, 32, 32], F16, tag="pm")
            nc.vector.tensor_scalar(pm[:], s1c[:, ::-1, :], sel1_sb[:, 0:1], None,
                                    op0=ALU.mult)
            pc = peerp.tile([128, 32, 32], F16, tag="pc")
            nc.vector.scalar_tensor_tensor(pc[:], s0c[:, ::-1, :], sel0_sb[:, 0:1],
                                           pm[:], op0=ALU.mult, op1=ALU.add)
            asm = gst.tile([128, 32, 128], F16, tag="asm")
            for m in range(8):
                ps = gps.tile([128, 512], F32, tag="gps")
                for k in range(4):
                    rhs = (ownc[:, :, ds(k * 16, 16)] if k < 2
                           else pc[:, :, ds((k - 2) * 16, 16)])
                    nc.tensor.matmul(ps[:], w1_sb[:, ds((k * 8 + m) * 128, 128)],
                                     rhs, start=(k == 0), stop=(k == 3))
                nc.vector.tensor_scalar(asm[:, :, ds(m * 16, 16)],
                                         ps.rearrange("p (t b) -> p t b", b=16),
                                         b1_sb[:, ds(m, 1)], None, op0=ALU.add)
            nc.sync.dma_start(xp1_d[:, ds(tci * 4096, 4096)], asm[:])

        tc.strict_bb_all_engine_barrier()
        # --- L1 scan
        _scan_range(tc, nc, st, 0, T // CH, u1_sb, xp1_d, own1_d, ident,
                    scan_pools, prologue=True)
        tc.strict_bb_all_engine_barrier()

        # --- head: y = h1_own @ wd_half (partial; host sums pairs)
        y_sb = const.tile([1, NT], F32, tag="ysb")
        for tci in range(T // 32):
            t0 = tci * 32
            own1c = peerp.tile([128, 32, 32], F16, tag="own1c")
            nc.sync.dma_start(own1c[:], own1_d[:, ds(32 * t0, 1024)]
                              .rearrange("p (t c) -> p t c", c=32))
            ps = hps.tile([1, 512], F32, tag="hps")
            for k in range(2):
                nc.tensor.matmul(ps[:], wd_sb[:, ds(k, 1)],
                                 own1c[:, :, ds(k * 16, 16)],
                                 start=(k == 0), stop=(k == 1))
            nc.vector.tensor_copy(y_sb[:, ds(tci * 512, 512)], ps[:])
        nc.sync.dma_start(y[:], y_sb[:])

    if finalize:
        nc.finalize()
    return nc


_NC_CACHE = {}


def _get_nc(T):
    if T not in _NC_CACHE:
        _NC_CACHE[T] = build_nc(T)
    return _NC_CACHE[T]


def kernel(**inputs) -> np.ndarray:
    T = np.asarray(inputs['x']).shape[1]
    in_maps = host_prep(inputs, T=T)
    nc = _get_nc(T)
    res = run_bass_kernel_spmd(nc, in_maps, core_ids=list(range(NCORES)))
    return host_post(res.results, inputs, T=T)


if __name__ == "__main__":
    rng = np.random.default_rng(0)
    Tt = 64
    demo = {
        'x': rng.standard_normal((B, Tt, F_IN), NP32),
        'Wd': rng.standard_normal((2 * H, 1), NP32) * 0.04,
        'bd': np.zeros(1, NP32),
    }
    for sfx in ('f', 'b'):
        demo[f'W0{sfx}'] = rng.standard_normal((F_IN, G4), NP32) * 0.125
        demo[f'U0{sfx}'] = rng.standard_normal((H, G4), NP32) * 0.0625
        demo[f'b0{sfx}'] = np.zeros(G4, NP32)
        demo[f'W1{sfx}'] = rng.standard_normal((2 * H, G4), NP32) * 0.044
        demo[f'U1{sfx}'] = rng.standard_normal((H, G4), NP32) * 0.0625
        demo[f'b1{sfx}'] = np.zeros(G4, NP32)
    out = kernel(**demo)
    print("kernel output:", out.shape, out.dtype, np.abs(out).max())


# revision 22
# speedup vs baseline: 1.2418x; 1.0006x over previous
"""Trainium2 Bass kernel for a 2-layer bidirectional LSTM + Dense(1) head.

Problem shapes: x [64, 1024, 64]; per layer/direction W [Fin, 1024], U [256, 1024],
b [1024]; head Wd [512, 1], bd [1]. Output [64, 1024, 1] fp32.

Sharding: 8 cores = 2 directions x 4 batch groups (16 rows per core). Each core
runs one scan per layer over its group. The fwd/bwd halves needed by layer 1 are
exchanged between core pairs with an AllGather; everything else is core-local.

Device-side math uses the all-sigmoid LSTM formulation:
    s = sigmoid(z_hat)                  (z_hat has tanh args pre-scaled by 2)
    c = s_f * c + 2*(s_g - 0.5)*s_i     (= f*c + i*tanh(z_g))
    h_stored = (sigmoid(2c) - 0.5)*s_o  (= o*tanh(c)/2, the /2 is folded into
                                         every weight that consumes h)
Weights/x/xp/h are fp16 on device (fp32 accumulation in PSUM); c and sigmoid
evaluations are fp32.
"""
import numpy as np
from contextlib import ExitStack

import concourse.bass as bass
import concourse.bacc as bacc
import concourse.mybir as mybir
import concourse.tile as tile
from concourse.bass import ds
from concourse.bass_utils import run_bass_kernel_spmd
from concourse.masks import make_identity

F16, F32 = mybir.dt.float16, mybir.dt.float32
NP16, NP32 = np.float16, np.float32

B, T_FULL, F_IN, H = 64, 1024, 64, 256
G4 = 4 * H          # 1024 gate columns
NCORES = 8
BL = 16             # batch rows per core
CH = 8              # scan steps per xp stream chunk
AF = mybir.ActivationFunctionType
ALU = mybir.AluOpType


# ---------------------------------------------------------------- host prep --

def _gate_perm():
    # reference gate col order [i, f, g, o] -> ours [g, i, f, o]
    return np.concatenate(
        [np.arange(2 * H, 3 * H), np.arange(0, H), np.arange(H, 2 * H),
         np.arange(3 * H, 4 * H)])


def _colscale():
    s = np.ones(G4, NP32)
    s[:H] = 2.0      # tanh trick: sigma(2*z_g); g block is first
    return s


def _prep_layer(Wref, Uref, bref, h_scaled_input):
    p, cs = _gate_perm(), _colscale()
    W = Wref[:, p] * cs[None, :]
    if h_scaled_input:
        W = W * 2.0
    U = Uref[:, p] * cs[None, :] * 2.0
    b = bref[p] * cs
    return W.astype(NP32), U.astype(NP32), b.astype(NP32)


def _tile_u(U):
    # [256, 1024] -> [128, 2048], col = (k*8+m)*128 + j
    return np.ascontiguousarray(
        U.reshape(2, 128, 8, 128).transpose(1, 0, 2, 3).reshape(128, 2048))


def _tile_w1(W):
    # [512, 1024] -> [128, 4096], col = (k*8+m)*128 + j
    return np.ascontiguousarray(
        W.reshape(4, 128, 8, 128).transpose(1, 0, 2, 3).reshape(128, 4096))


def host_prep(inputs, T=T_FULL, ncores=NCORES):
    """Returns list of per-core input dicts for run_bass_kernel_spmd."""
    x = np.asarray(inputs['x'])[:, :T, :]
    Wd = np.asarray(inputs['Wd'])

    Wh = {}
    for d, sfx in (('f', 'f'), ('b', 'b')):
        W0, U0, b0 = _prep_layer(np.asarray(inputs['W0' + sfx]),
                                 np.asarray(inputs['U0' + sfx]),
                                 np.asarray(inputs['b0' + sfx]), False)
        W1, U1, b1 = _prep_layer(np.asarray(inputs['W1' + sfx]),
                                 np.asarray(inputs['U1' + sfx]),
                                 np.asarray(inputs['b1' + sfx]), True)
        Wh[d] = (W0, U0, b0, W1, U1, b1)

    in_maps = []
    for c in range(ncores):
        d = 'f' if c % 2 == 0 else 'b'
        g = c // 2
        W0, U0, b0, W1, U1, b1 = Wh[d]

        xg = np.transpose(x[g * BL:(g + 1) * BL], (1, 0, 2))  # [T, BL, F]
        if d == 'b':
            xg = xg[::-1]
        # xT [65, 16T]: rows = feature, col = t*16+b; last row = ones
        xT = np.transpose(xg, (2, 0, 1)).reshape(F_IN, T * BL)
        xT = np.concatenate([xT, np.ones((1, T * BL), NP32)], 0).astype(NP16)

        # w0 aug with bias row -> [65, 1024]
        w0 = np.concatenate([W0, b0[None, :]], 0).astype(NP16)

        W1h = W1
        if d == 'b':
            W1h = np.concatenate([W1[H:2 * H], W1[:H]], 0)  # own-dir rows first
        wdh = (2.0 * Wd[:H, 0]) if d == 'f' else (2.0 * Wd[H:, 0])

        sel0 = 1.0 if c % 2 == 1 else 0.0   # peer slot: even core's peer is slot1
        in_maps.append({
            'xT': np.ascontiguousarray(xT),
            'u0': _tile_u(U0).astype(NP16),
            'w0': np.ascontiguousarray(w0),
            'u1': _tile_u(U1).astype(NP16),
            'w1': _tile_w1(W1h).astype(NP16),
            'b1': np.ascontiguousarray(b1.reshape(8, 128).T).astype(NP32),
            'wd': np.ascontiguousarray(wdh.reshape(2, 128).T).astype(NP16),
            'sel0': np.full((128, 1), sel0, NP32),
            'sel1': np.full((128, 1), 1.0 - sel0, NP32),
        })
    return in_maps


def host_post(results, inputs, T=T_FULL):
    bd = np.asarray(inputs['bd'])
    y = np.zeros((B, T, 1), NP32)
    for g in range(NCORES // 2):
        yf = results[2 * g]['y'].reshape(T, BL)
        yb = results[2 * g + 1]['y'].reshape(T, BL)[::-1]
        y[g * BL:(g + 1) * BL, :, 0] = (yf + yb).T + bd[0]
    return y


# ------------------------------------------------------------- device build --

def _scan_state(nc, pools):
    """Allocate scan state tiles (shared across sub-loops and layers)."""
    zp, sp, wp, cst, xpp = pools
    st = {}
    st['cA'] = cst.tile([128, 32], F32, tag="cA", name="cA")
    st['cB'] = cst.tile([128, 32], F32, tag="cB", name="cB")
    st['stgA'] = cst.tile([128, CH * 32], F16, tag="stgA", name="stgA")
    st['stgB'] = cst.tile([128, CH * 32], F16, tag="stgB", name="stgB")
    st['xpA'] = xpp.tile([128, CH, 128], F16, tag="xpA", name="xpA")
    st['xpB'] = xpp.tile([128, CH, 128], F16, tag="xpB", name="xpB")
    return st


def _scan_range(tc, nc, st, lo, hi, u_sb, xp_d, hst_d, ident, pools, prologue, col0=0):
    """Emit scan steps for chunk range [lo, hi) (each chunk = CH steps).

    All compute APs are static; h history lives in two chunk staging tiles
    (stgA/stgB) and is appended to the DRAM store hst_d via DMA (dynamic
    offsets are DRAM-side only). hst_d col block s = h(s) at [32s, 32s+32).
    """
    zp, sp, wp, cst, xpp = pools
    cA, cB = st['cA'], st['cB']
    stgA, stgB, xpA, xpB = st['stgA'], st['stgB'], st['xpA'], st['xpB']
    if prologue:
        nc.vector.memset(cA[:], 0.0)
        nc.vector.memset(stgB[:, (CH - 1) * 32:], 0.0)   # h(-1) = 0
        nc.sync.dma_start(xpA[:], xp_d[:, ds(0, CH * 128)])

    def step(j, xp_tile, stg, stg_prev, c_in, c_out):
        h_prev = (stg_prev[:, (CH - 1) * 32:] if j == 0
                  else stg[:, (j - 1) * 32:j * 32])
        zt = zp.tile([128, 128], F32, tag="z")
        nc.tensor.matmul(zt[:], ident[:], xp_tile[:, j, :],
                         start=True, stop=False, skip_group_check=True)
        for k in range(2):
            for m in range(8):
                nc.tensor.matmul(
                    zt[:, ds(m * 16, 16)],
                    u_sb[:, ds((k * 8 + m) * 128, 128)],
                    h_prev[:, ds(k * 16, 16)],
                    start=False, stop=(m == 7 and k == 1), skip_group_check=True)
        # gate col order [g, i, f, o]; cell state kept as c' = c/2 so the
        # update is a plain add: c' = (s_g-.5)*s_i + s_f*c'
        s_t = sp.tile([128, 128], F16, tag="s")
        nc.scalar.activation(s_t[:], zt[:], AF.Sigmoid)
        t1 = wp.tile([128, 32], F16, tag="t1")
        nc.vector.scalar_tensor_tensor(t1[:], s_t[:, 0:32], -0.5, s_t[:, 32:64],
                                       op0=ALU.add, op1=ALU.mult)
        tm = wp.tile([128, 32], F32, tag="tm")
        nc.vector.tensor_mul(tm[:], s_t[:, 64:96], c_in[:])
        nc.vector.tensor_add(c_out[:], t1[:], tm[:])
        sc = wp.tile([128, 32], F16, tag="sc")
        nc.scalar.activation(sc[:], c_out[:], AF.Sigmoid, scale=4.0)
        nc.vector.scalar_tensor_tensor(
            stg[:, j * 32:(j + 1) * 32],
            sc[:], -0.5, s_t[:, 96:128], op0=ALU.add, op1=ALU.mult)

    def chunk(xp_tile, stg, stg_prev):
        for j in range(CH):
            even = (j % 2 == 0)
            step(j, xp_tile, stg, stg_prev, cA if even else cB, cB if even else cA)

    with tc.For_i(lo, hi, 2, staggered_reset=True,
                  hint_engines=(mybir.EngineType.PE, mybir.EngineType.DVE,
                                mybir.EngineType.Activation)) as i:
        nc.sync.dma_start(xpB[:], xp_d[:, ds(i * (CH * 128) + CH * 128, CH * 128)])
        chunk(xpA, stgA, stgB)
        nc.sync.dma_start(hst_d[:, ds(i * (CH * 32) - col0, CH * 32)], stgA[:])
        nc.sync.dma_start(xpA[:], xp_d[:, ds(i * (CH * 128) + 2 * CH * 128, CH * 128)])
        chunk(xpB, stgB, stgA)
        nc.sync.dma_start(hst_d[:, ds(i * (CH * 32) + CH * 32 - col0, CH * 32)], stgB[:])


def build_nc(T=T_FULL, pad_init=False, finalize=True):
    nc = bacc.Bacc(None, num_devices=NCORES)
    NT = BL * T

    xT = nc.declare_dram_parameter("xT", [F_IN + 1, NT], F16, isOutput=False)
    u0 = nc.declare_dram_parameter("u0", [128, 2048], F16, isOutput=False)
    w0 = nc.declare_dram_parameter("w0", [F_IN + 1, 1024], F16, isOutput=False)
    u1 = nc.declare_dram_parameter("u1", [128, 2048], F16, isOutput=False)
    w1 = nc.declare_dram_parameter("w1", [128, 4096], F16, isOutput=False)
    b1 = nc.declare_dram_parameter("b1", [128, 8], F32, isOutput=False)
    wd = nc.declare_dram_parameter("wd", [128, 2], F16, isOutput=False)
    sel0 = nc.declare_dram_parameter("sel0", [128, 1], F32, isOutput=False)
    sel1 = nc.declare_dram_parameter("sel1", [128, 1], F32, isOutput=False)
    y = nc.declare_dram_parameter("y", [1, NT], F32, isOutput=True)

    xp0_d = nc.dram_tensor("xp0_d", [128, (T + 2 * CH) * 128], F16)
    xp1_d = nc.dram_tensor("xp1_d", [128, (T + 2 * CH) * 128], F16)
    NQ = 4 if (T % 128 == 0 and (T // CH // 4) % 2 == 0) else 1
    qcols = 32 * T // NQ
    exch_in = nc.dram_tensor("exch_in", [NQ, 128, qcols], F16)
    own1_d = nc.dram_tensor("own1_d", [128, 32 * T], F16)
    # note: addr_space="Shared" is rejected for 2-core groups; Local works.
    exch_out = nc.dram_tensor("exch_out", [NQ, 2, 128, qcols], F16)

    with tile.TileContext(nc) as tc, ExitStack() as ctx:
        const = ctx.enter_context(tc.tile_pool(name="const", bufs=1))
        xpp = ctx.enter_context(tc.tile_pool(name="xpp", bufs=1))
        gst = ctx.enter_context(tc.tile_pool(name="gst", bufs=3))
        peerp = ctx.enter_context(tc.tile_pool(name="peerp", bufs=2))
        sp = ctx.enter_context(tc.tile_pool(name="sp", bufs=3))
        wp = ctx.enter_context(tc.tile_pool(name="wp", bufs=3))
        cst = ctx.enter_context(tc.tile_pool(name="cst", bufs=1))
        zp = ctx.enter_context(tc.tile_pool(name="zp", bufs=2, space="PSUM"))
        gps = ctx.enter_context(tc.tile_pool(name="gps", bufs=4, space="PSUM"))
        hps = ctx.enter_context(tc.tile_pool(name="hps", bufs=2, space="PSUM"))

        # --- load parameters to SBUF
        u0_sb = const.tile([128, 2048], F16, tag="u0")
        nc.sync.dma_start(u0_sb[:], u0[:])
        u1_sb = const.tile([128, 2048], F16, tag="u1")
        nc.sync.dma_start(u1_sb[:], u1[:])
        w0_sb = const.tile([F_IN + 1, 1024], F16, tag="w0")
        nc.sync.dma_start(w0_sb[:], w0[:])
        w1_sb = const.tile([128, 4096], F16, tag="w1")
        nc.sync.dma_start(w1_sb[:], w1[:])
        b1_sb = const.tile([128, 8], F32, tag="b1")
        nc.sync.dma_start(b1_sb[:], b1[:])
        wd_sb = const.tile([128, 2], F16, tag="wd")
        nc.sync.dma_start(wd_sb[:], wd[:])
        sel0_sb = const.tile([128, 1], F32, tag="sel0")
        nc.sync.dma_start(sel0_sb[:], sel0[:])
        sel1_sb = const.tile([128, 1], F32, tag="sel1")
        nc.sync.dma_start(sel1_sb[:], sel1[:])
        xT_sb = const.tile([F_IN + 1, NT], F16, tag="xT")
        nc.sync.dma_start(xT_sb[:], xT[:])
        ident = const.tile([128, 128], F16, tag="ident")
        make_identity(nc, ident[:])
        warm = const.tile([128, 1], F32, tag="warm")
        nc.vector.memset(warm[:], 0.0)
        nc.scalar.activation(warm[:], warm[:], AF.Sigmoid)
        if pad_init:
            # Only to satisfy the simulator's NaN-canary on the prefetch
            # overrun region; the values are never consumed by compute.
            zpad = const.tile([128, 2 * CH * 128], F16, tag="zpad")
            nc.vector.memset(zpad[:], 0.0)
            nc.sync.dma_start(xp0_d[:, ds(T * 128, 2 * CH * 128)], zpad[:])
            nc.sync.dma_start(xp1_d[:, ds(T * 128, 2 * CH * 128)], zpad[:])

        scan_pools = (zp, sp, wp, cst, xpp)

        # --- xp0 = [x;1] @ [W0;b0]  -> xp0_d (t-blocked layout)
        for tci in range(T // 32):
            asm = gst.tile([128, 32, 128], F16, tag="asm")
            for m in range(8):
                ps = gps.tile([128, 512], F32, tag="gps")
                nc.tensor.matmul(ps[:], w0_sb[:, ds(m * 128, 128)],
                                 xT_sb[:, ds(tci * 512, 512)],
                                 start=True, stop=True)
                if m % 2 == 0:
                    nc.vector.tensor_copy(asm[:, :, ds(m * 16, 16)],
                                          ps.rearrange("p (t b) -> p t b", b=16))
                else:
                    nc.scalar.copy(asm[:, :, ds(m * 16, 16)],
                                   ps.rearrange("p (t b) -> p t b", b=16))
            nc.sync.dma_start(xp0_d[:, ds(tci * 4096, 4096)], asm[:])

        tc.strict_bb_all_engine_barrier()
        # --- L0 scan in quarters; exchange each quarter as it completes
        rgroups = [[2 * g, 2 * g + 1] for g in range(NCORES // 2)]
        st = _scan_state(nc, scan_pools)
        qchunks = T // CH // NQ
        ccs = []
        for q in range(NQ):
            _scan_range(tc, nc, st, q * qchunks, (q + 1) * qchunks,
                        u0_sb, xp0_d, exch_in[q], ident, scan_pools,
                        prologue=(q == 0), col0=q * qcols)
            cc = nc.gpsimd.collective_compute(
                "AllGather", ALU.bypass, replica_groups=rgroups,
                ins=[exch_in[q]], outs=[exch_out[q]])
            ccs.append(cc)

        # --- xp1 = [own; peer_reversed] @ W1 + b1 -> xp1_d
        # tci descending so the first-needed peer quarters are the
        # first-exchanged ones; peer DMAs gate on their quarter's collective.
        for tci in range(T // 32 - 1, -1, -1):
            t0 = tci * 32
            q = (32 * (T - 32 - t0)) // qcols
            qoff = 32 * (T - 32 - t0) - q * qcols
            q_own = (32 * t0) // qcols
            ownc = peerp.tile([128, 32, 32], F16, tag="ownc")
            do = nc.sync.dma_start(
                ownc[:], exch_in[q_own, :, ds(32 * t0 - q_own * qcols, 1024)]
                .rearrange("p (t c) -> p t c", c=32))
            tile.add_dep_helper(do.ins, ccs[q_own].ins, reason="own q written")
            s0c = peerp.tile([128, 32, 32], F16, tag="s0c")
            d0 = nc.sync.dma_start(s0c[:],
                                   exch_out[q, 0, :, ds(qoff, 1024)]
                                   .rearrange("p (t c) -> p t c", c=32))
            s1c = peerp.tile([128, 32, 32], F16, tag="s1c")
            d1 = nc.sync.dma_start(s1c[:],
                                   exch_out[q, 1, :, ds(qoff, 1024)]
                                   .rearrange("p (t c) -> p t c", c=32))
            tile.add_dep_helper(d0.ins, ccs[q].ins, reason="peer q ready")
            tile.add_dep_helper(d1.ins, ccs[q].ins, reason="peer q ready")
            pm = peerp.tile([128, 32, 32], F16, tag="pm")
            nc.vector.tensor_scalar(pm[:], s1c[:, ::-1, :], sel1_sb[:, 0:1], None,
                                    op0=ALU.mult)
            pc = peerp.tile([128, 32, 32], F16, tag="pc")
            nc.vector.scalar_tensor_tensor(pc[:], s0c[:, ::-1, :], sel0_sb[:, 0:1],
                                           pm[:], op0=ALU.mult, op1=ALU.add)
            asm = gst.tile([128, 32, 128], F16, tag="asm")
            for m in range(8):
                ps = gps.tile([128, 512], F32, tag="gps")
                for k in range(4):
                    rhs = (ownc[:, :, ds(k * 16, 16)] if k < 2
                           else pc[:, :, ds((k - 2) * 16, 16)])
                    nc.tensor.matmul(ps[:], w1_sb[:, ds((k * 8 + m) * 128, 128)],
                                     rhs, start=(k == 0), stop=(k == 3))
                nc.vector.tensor_scalar(asm[:, :, ds(m * 16, 16)],
                                         ps.rearrange("p (t b) -> p t b", b=16),
                                         b1_sb[:, ds(m, 1)], None, op0=ALU.add)
            nc.sync.dma_start(xp1_d[:, ds(tci * 4096, 4096)], asm[:])

        tc.strict_bb_all_engine_barrier()
        # --- L1 scan
        _scan_range(tc, nc, st, 0, T // CH, u1_sb, xp1_d, own1_d, ident,
                    scan_pools, prologue=True)
        tc.strict_bb_all_engine_barrier()

        # --- head: y = h1_own @ wd_half (partial; host sums pairs)
        y_sb = const.tile([1, NT], F32, tag="ysb")
        for tci in range(T // 32):
            t0 = tci * 32
            own1c = peerp.tile([128, 32, 32], F16, tag="own1c")
            nc.sync.dma_start(own1c[:], own1_d[:, ds(32 * t0, 1024)]
                              .rearrange("p (t c) -> p t c", c=32))
            ps = hps.tile([1, 512], F32, tag="hps")
            for k in range(2):
                nc.tensor.matmul(ps[:], wd_sb[:, ds(k, 1)],
                                 own1c[:, :, ds(k * 16, 16)],
                                 start=(k == 0), stop=(k == 1))
            nc.vector.tensor_copy(y_sb[:, ds(tci * 512, 512)], ps[:])
        nc.sync.dma_start(y[:], y_sb[:])

    if finalize:
        nc.finalize()
    return nc


_NC_CACHE = {}


def _get_nc(T):
    if T not in _NC_CACHE:
        _NC_CACHE[T] = build_nc(T)
    return _NC_CACHE[T]


def kernel(**inputs) -> np.ndarray:
    T = np.asarray(inputs['x']).shape[1]
    in_maps = host_prep(inputs, T=T)
    nc = _get_nc(T)
    res = run_bass_kernel_spmd(nc, in_maps, core_ids=list(range(NCORES)))
    return host_post(res.results, inputs, T=T)


if __name__ == "__main__":
    rng = np.random.default_rng(0)
    Tt = 64
    demo = {
        'x': rng.standard_normal((B, Tt, F_IN), NP32),
        'Wd': rng.standard_normal((2 * H, 1), NP32) * 0.04,
        'bd': np.zeros(1, NP32),
    }
    for sfx in ('f', 'b'):
        demo[f'W0{sfx}'] = rng.standard_normal((F_IN, G4), NP32) * 0.125
        demo[f'U0{sfx}'] = rng.standard_normal((H, G4), NP32) * 0.0625
        demo[f'b0{sfx}'] = np.zeros(G4, NP32)
        demo[f'W1{sfx}'] = rng.standard_normal((2 * H, G4), NP32) * 0.044
        demo[f'U1{sfx}'] = rng.standard_normal((H, G4), NP32) * 0.0625
        demo[f'b1{sfx}'] = np.zeros(G4, NP32)
    out = kernel(**demo)
    print("kernel output:", out.shape, out.dtype, np.abs(out).max())


# revision 23
# speedup vs baseline: 1.2422x; 1.0003x over previous
"""Trainium2 Bass kernel for a 2-layer bidirectional LSTM + Dense(1) head.

Problem shapes: x [64, 1024, 64]; per layer/direction W [Fin, 1024], U [256, 1024],
b [1024]; head Wd [512, 1], bd [1]. Output [64, 1024, 1] fp32.

Sharding: 8 cores = 2 directions x 4 batch groups (16 rows per core). Each core
runs one scan per layer over its group. The fwd/bwd halves needed by layer 1 are
exchanged between core pairs with an AllGather; everything else is core-local.

Device-side math uses the all-sigmoid LSTM formulation:
    s = sigmoid(z_hat)                  (z_hat has tanh args pre-scaled by 2)
    c = s_f * c + 2*(s_g - 0.5)*s_i     (= f*c + i*tanh(z_g))
    h_stored = (sigmoid(2c) - 0.5)*s_o  (= o*tanh(c)/2, the /2 is folded into
                                         every weight that consumes h)
Weights/x/xp/h are fp16 on device (fp32 accumulation in PSUM); c and sigmoid
evaluations are fp32.
"""
import numpy as np
from contextlib import ExitStack

import concourse.bass as bass
import concourse.bacc as bacc
import concourse.mybir as mybir
import concourse.tile as tile
from concourse.bass import ds
from concourse.bass_utils import run_bass_kernel_spmd
from concourse.masks import make_identity

F16, F32 = mybir.dt.float16, mybir.dt.float32
NP16, NP32 = np.float16, np.float32

B, T_FULL, F_IN, H = 64, 1024, 64, 256
G4 = 4 * H          # 1024 gate columns
NCORES = 8
BL = 16             # batch rows per core
CH = 8              # scan steps per xp stream chunk
AF = mybir.ActivationFunctionType
ALU = mybir.AluOpType


# ---------------------------------------------------------------- host prep --

def _gate_perm():
    # reference gate col order [i, f, g, o] -> ours [g, i, f, o]
    return np.concatenate(
        [np.arange(2 * H, 3 * H), np.arange(0, H), np.arange(H, 2 * H),
         np.arange(3 * H, 4 * H)])


def _colscale():
    s = np.ones(G4, NP32)
    s[:H] = 2.0      # tanh trick: sigma(2*z_g); g block is first
    return s


def _prep_layer(Wref, Uref, bref, h_scaled_input):
    p, cs = _gate_perm(), _colscale()
    W = Wref[:, p] * cs[None, :]
    if h_scaled_input:
        W = W * 2.0
    U = Uref[:, p] * cs[None, :] * 2.0
    b = bref[p] * cs
    return W.astype(NP32), U.astype(NP32), b.astype(NP32)


def _tile_u(U):
    # [256, 1024] -> [128, 2048], col = (k*8+m)*128 + j
    return np.ascontiguousarray(
        U.reshape(2, 128, 8, 128).transpose(1, 0, 2, 3).reshape(128, 2048))


def _tile_w1(W):
    # [512, 1024] -> [128, 4096], col = (k*8+m)*128 + j
    return np.ascontiguousarray(
        W.reshape(4, 128, 8, 128).transpose(1, 0, 2, 3).reshape(128, 4096))


def host_prep(inputs, T=T_FULL, ncores=NCORES):
    """Returns list of per-core input dicts for run_bass_kernel_spmd."""
    x = np.asarray(inputs['x'])[:, :T, :]
    Wd = np.asarray(inputs['Wd'])

    Wh = {}
    for d, sfx in (('f', 'f'), ('b', 'b')):
        W0, U0, b0 = _prep_layer(np.asarray(inputs['W0' + sfx]),
                                 np.asarray(inputs['U0' + sfx]),
                                 np.asarray(inputs['b0' + sfx]), False)
        W1, U1, b1 = _prep_layer(np.asarray(inputs['W1' + sfx]),
                                 np.asarray(inputs['U1' + sfx]),
                                 np.asarray(inputs['b1' + sfx]), True)
        Wh[d] = (W0, U0, b0, W1, U1, b1)

    in_maps = []
    for c in range(ncores):
        d = 'f' if c % 2 == 0 else 'b'
        g = c // 2
        W0, U0, b0, W1, U1, b1 = Wh[d]

        xg = np.transpose(x[g * BL:(g + 1) * BL], (1, 0, 2))  # [T, BL, F]
        if d == 'b':
            xg = xg[::-1]
        # xT [65, 16T]: rows = feature, col = t*16+b; last row = ones
        xT = np.transpose(xg, (2, 0, 1)).reshape(F_IN, T * BL)
        xT = np.concatenate([xT, np.ones((1, T * BL), NP32)], 0).astype(NP16)

        # w0 aug with bias row -> [65, 1024]
        w0 = np.concatenate([W0, b0[None, :]], 0).astype(NP16)

        W1h = W1
        if d == 'b':
            W1h = np.concatenate([W1[H:2 * H], W1[:H]], 0)  # own-dir rows first
        wdh = (2.0 * Wd[:H, 0]) if d == 'f' else (2.0 * Wd[H:, 0])

        sel0 = 1.0 if c % 2 == 1 else 0.0   # peer slot: even core's peer is slot1
        in_maps.append({
            'xT': np.ascontiguousarray(xT),
            'u0': _tile_u(U0).astype(NP16),
            'w0': np.ascontiguousarray(w0),
            'u1': _tile_u(U1).astype(NP16),
            'w1': _tile_w1(W1h).astype(NP16),
            'b1': np.ascontiguousarray(b1.reshape(8, 128).T).astype(NP32),
            'wd': np.ascontiguousarray(wdh.reshape(2, 128).T).astype(NP16),
            'sel0': np.full((128, 1), sel0, NP32),
            'sel1': np.full((128, 1), 1.0 - sel0, NP32),
        })
    return in_maps


def host_post(results, inputs, T=T_FULL):
    bd = np.asarray(inputs['bd'])
    y = np.zeros((B, T, 1), NP32)
    for g in range(NCORES // 2):
        yf = results[2 * g]['y'].reshape(T, BL)
        yb = results[2 * g + 1]['y'].reshape(T, BL)[::-1]
        y[g * BL:(g + 1) * BL, :, 0] = (yf + yb).T + bd[0]
    return y


# ------------------------------------------------------------- device build --

def _scan_state(nc, pools):
    """Allocate scan state tiles (shared across sub-loops and layers)."""
    zp, sp, wp, cst, xpp = pools
    st = {}
    st['cA'] = cst.tile([128, 32], F32, tag="cA", name="cA")
    st['cB'] = cst.tile([128, 32], F32, tag="cB", name="cB")
    st['stgA'] = cst.tile([128, CH * 32], F16, tag="stgA", name="stgA")
    st['stgB'] = cst.tile([128, CH * 32], F16, tag="stgB", name="stgB")
    st['xpA'] = xpp.tile([128, CH, 128], F16, tag="xpA", name="xpA")
    st['xpB'] = xpp.tile([128, CH, 128], F16, tag="xpB", name="xpB")
    return st


def _scan_range(tc, nc, st, lo, hi, u_sb, xp_d, hst_d, ident, pools, prologue, col0=0):
    """Emit scan steps for chunk range [lo, hi) (each chunk = CH steps).

    All compute APs are static; h history lives in two chunk staging tiles
    (stgA/stgB) and is appended to the DRAM store hst_d via DMA (dynamic
    offsets are DRAM-side only). hst_d col block s = h(s) at [32s, 32s+32).
    """
    zp, sp, wp, cst, xpp = pools
    cA, cB = st['cA'], st['cB']
    stgA, stgB, xpA, xpB = st['stgA'], st['stgB'], st['xpA'], st['xpB']
    if prologue:
        nc.vector.memset(cA[:], 0.0)
        nc.vector.memset(stgB[:, (CH - 1) * 32:], 0.0)   # h(-1) = 0
        nc.sync.dma_start(xpA[:], xp_d[:, ds(0, CH * 128)])

    def step(j, xp_tile, stg, stg_prev, c_in, c_out):
        h_prev = (stg_prev[:, (CH - 1) * 32:] if j == 0
                  else stg[:, (j - 1) * 32:j * 32])
        zt = zp.tile([128, 128], F32, tag="z")
        nc.tensor.matmul(zt[:], ident[:], xp_tile[:, j, :],
                         start=True, stop=False, skip_group_check=True)
        for k in range(2):
            for m in range(8):
                nc.tensor.matmul(
                    zt[:, ds(m * 16, 16)],
                    u_sb[:, ds((k * 8 + m) * 128, 128)],
                    h_prev[:, ds(k * 16, 16)],
                    start=False, stop=(m == 7 and k == 1), skip_group_check=True)
        # gate col order [g, i, f, o]; cell state kept as c' = c/2 so the
        # update is a plain add: c' = (s_g-.5)*s_i + s_f*c'
        s_t = sp.tile([128, 128], F32, tag="s")
        nc.scalar.activation(s_t[:], zt[:], AF.Sigmoid)
        t1 = wp.tile([128, 32], F32, tag="t1")
        nc.vector.scalar_tensor_tensor(t1[:], s_t[:, 0:32], -0.5, s_t[:, 32:64],
                                       op0=ALU.add, op1=ALU.mult)
        tm = wp.tile([128, 32], F32, tag="tm")
        nc.vector.tensor_mul(tm[:], s_t[:, 64:96], c_in[:])
        nc.vector.tensor_add(c_out[:], t1[:], tm[:])
        sc = wp.tile([128, 32], F32, tag="sc")
        nc.scalar.activation(sc[:], c_out[:], AF.Sigmoid, scale=4.0)
        nc.vector.scalar_tensor_tensor(
            stg[:, j * 32:(j + 1) * 32],
            sc[:], -0.5, s_t[:, 96:128], op0=ALU.add, op1=ALU.mult)

    def chunk(xp_tile, stg, stg_prev):
        for j in range(CH):
            even = (j % 2 == 0)
            step(j, xp_tile, stg, stg_prev, cA if even else cB, cB if even else cA)

    with tc.For_i(lo, hi, 2, staggered_reset=True,
                  hint_engines=(mybir.EngineType.PE, mybir.EngineType.DVE,
                                mybir.EngineType.Activation)) as i:
        nc.sync.dma_start(xpB[:], xp_d[:, ds(i * (CH * 128) + CH * 128, CH * 128)])
        chunk(xpA, stgA, stgB)
        nc.sync.dma_start(hst_d[:, ds(i * (CH * 32) - col0, CH * 32)], stgA[:])
        nc.sync.dma_start(xpA[:], xp_d[:, ds(i * (CH * 128) + 2 * CH * 128, CH * 128)])
        chunk(xpB, stgB, stgA)
        nc.sync.dma_start(hst_d[:, ds(i * (CH * 32) + CH * 32 - col0, CH * 32)], stgB[:])


def build_nc(T=T_FULL, pad_init=False, finalize=True):
    nc = bacc.Bacc(None, num_devices=NCORES)
    NT = BL * T

    xT = nc.declare_dram_parameter("xT", [F_IN + 1, NT], F16, isOutput=False)
    u0 = nc.declare_dram_parameter("u0", [128, 2048], F16, isOutput=False)
    w0 = nc.declare_dram_parameter("w0", [F_IN + 1, 1024], F16, isOutput=False)
    u1 = nc.declare_dram_parameter("u1", [128, 2048], F16, isOutput=False)
    w1 = nc.declare_dram_parameter("w1", [128, 4096], F16, isOutput=False)
    b1 = nc.declare_dram_parameter("b1", [128, 8], F32, isOutput=False)
    wd = nc.declare_dram_parameter("wd", [128, 2], F16, isOutput=False)
    sel0 = nc.declare_dram_parameter("sel0", [128, 1], F32, isOutput=False)
    sel1 = nc.declare_dram_parameter("sel1", [128, 1], F32, isOutput=False)
    y = nc.declare_dram_parameter("y", [1, NT], F32, isOutput=True)

    xp0_d = nc.dram_tensor("xp0_d", [128, (T + 2 * CH) * 128], F16)
    xp1_d = nc.dram_tensor("xp1_d", [128, (T + 2 * CH) * 128], F16)
    NQ = 4 if (T % 128 == 0 and (T // CH // 4) % 2 == 0) else 1
    qcols = 32 * T // NQ
    exch_in = nc.dram_tensor("exch_in", [NQ, 128, qcols], F16)
    own1_d = nc.dram_tensor("own1_d", [128, 32 * T], F16)
    # note: addr_space="Shared" is rejected for 2-core groups; Local works.
    exch_out = nc.dram_tensor("exch_out", [NQ, 2, 128, qcols], F16)

    with tile.TileContext(nc) as tc, ExitStack() as ctx:
        const = ctx.enter_context(tc.tile_pool(name="const", bufs=1))
        xpp = ctx.enter_context(tc.tile_pool(name="xpp", bufs=1))
        gst = ctx.enter_context(tc.tile_pool(name="gst", bufs=3))
        peerp = ctx.enter_context(tc.tile_pool(name="peerp", bufs=2))
        sp = ctx.enter_context(tc.tile_pool(name="sp", bufs=3))
        wp = ctx.enter_context(tc.tile_pool(name="wp", bufs=3))
        cst = ctx.enter_context(tc.tile_pool(name="cst", bufs=1))
        zp = ctx.enter_context(tc.tile_pool(name="zp", bufs=2, space="PSUM"))
        gps = ctx.enter_context(tc.tile_pool(name="gps", bufs=4, space="PSUM"))
        hps = ctx.enter_context(tc.tile_pool(name="hps", bufs=2, space="PSUM"))

        # --- load parameters to SBUF
        u0_sb = const.tile([128, 2048], F16, tag="u0")
        nc.sync.dma_start(u0_sb[:], u0[:])
        u1_sb = const.tile([128, 2048], F16, tag="u1")
        nc.sync.dma_start(u1_sb[:], u1[:])
        w0_sb = const.tile([F_IN + 1, 1024], F16, tag="w0")
        nc.sync.dma_start(w0_sb[:], w0[:])
        w1_sb = const.tile([128, 4096], F16, tag="w1")
        nc.sync.dma_start(w1_sb[:], w1[:])
        b1_sb = const.tile([128, 8], F32, tag="b1")
        nc.sync.dma_start(b1_sb[:], b1[:])
        wd_sb = const.tile([128, 2], F16, tag="wd")
        nc.sync.dma_start(wd_sb[:], wd[:])
        sel0_sb = const.tile([128, 1], F32, tag="sel0")
        nc.sync.dma_start(sel0_sb[:], sel0[:])
        sel1_sb = const.tile([128, 1], F32, tag="sel1")
        nc.sync.dma_start(sel1_sb[:], sel1[:])
        xT_sb = const.tile([F_IN + 1, NT], F16, tag="xT")
        nc.sync.dma_start(xT_sb[:], xT[:])
        ident = const.tile([128, 128], F16, tag="ident")
        make_identity(nc, ident[:])
        warm = const.tile([128, 1], F32, tag="warm")
        nc.vector.memset(warm[:], 0.0)
        nc.scalar.activation(warm[:], warm[:], AF.Sigmoid)
        if pad_init:
            # Only to satisfy the simulator's NaN-canary on the prefetch
            # overrun region; the values are never consumed by compute.
            zpad = const.tile([128, 2 * CH * 128], F16, tag="zpad")
            nc.vector.memset(zpad[:], 0.0)
            nc.sync.dma_start(xp0_d[:, ds(T * 128, 2 * CH * 128)], zpad[:])
            nc.sync.dma_start(xp1_d[:, ds(T * 128, 2 * CH * 128)], zpad[:])

        scan_pools = (zp, sp, wp, cst, xpp)

        # --- xp0 = [x;1] @ [W0;b0]  -> xp0_d (t-blocked layout)
        for tci in range(T // 32):
            asm = gst.tile([128, 32, 128], F16, tag="asm")
            for m in range(8):
                ps = gps.tile([128, 512], F32, tag="gps")
                nc.tensor.matmul(ps[:], w0_sb[:, ds(m * 128, 128)],
                                 xT_sb[:, ds(tci * 512, 512)],
                                 start=True, stop=True)
                if m % 2 == 0:
                    nc.vector.tensor_copy(asm[:, :, ds(m * 16, 16)],
                                          ps.rearrange("p (t b) -> p t b", b=16))
                else:
                    nc.scalar.copy(asm[:, :, ds(m * 16, 16)],
                                   ps.rearrange("p (t b) -> p t b", b=16))
            nc.sync.dma_start(xp0_d[:, ds(tci * 4096, 4096)], asm[:])

        tc.strict_bb_all_engine_barrier()
        # --- L0 scan in quarters; exchange each quarter as it completes
        rgroups = [[2 * g, 2 * g + 1] for g in range(NCORES // 2)]
        st = _scan_state(nc, scan_pools)
        qchunks = T // CH // NQ
        ccs = []
        for q in range(NQ):
            _scan_range(tc, nc, st, q * qchunks, (q + 1) * qchunks,
                        u0_sb, xp0_d, exch_in[q], ident, scan_pools,
                        prologue=(q == 0), col0=q * qcols)
            cc = nc.gpsimd.collective_compute(
                "AllGather", ALU.bypass, replica_groups=rgroups,
                ins=[exch_in[q]], outs=[exch_out[q]])
            ccs.append(cc)

        # --- xp1 = [own; peer_reversed] @ W1 + b1 -> xp1_d
        # tci descending so the first-needed peer quarters are the
        # first-exchanged ones; peer DMAs gate on their quarter's collective.
        for tci in range(T // 32 - 1, -1, -1):
            t0 = tci * 32
            q = (32 * (T - 32 - t0)) // qcols
            qoff = 32 * (T - 32 - t0) - q * qcols
            q_own = (32 * t0) // qcols
            ownc = peerp.tile([128, 32, 32], F16, tag="ownc")
            do = nc.sync.dma_start(
                ownc[:], exch_in[q_own, :, ds(32 * t0 - q_own * qcols, 1024)]
                .rearrange("p (t c) -> p t c", c=32))
            tile.add_dep_helper(do.ins, ccs[q_own].ins, reason="own q written")
            s0c = peerp.tile([128, 32, 32], F16, tag="s0c")
            d0 = nc.sync.dma_start(s0c[:],
                                   exch_out[q, 0, :, ds(qoff, 1024)]
                                   .rearrange("p (t c) -> p t c", c=32))
            s1c = peerp.tile([128, 32, 32], F16, tag="s1c")
            d1 = nc.sync.dma_start(s1c[:],
                                   exch_out[q, 1, :, ds(qoff, 1024)]
                                   .rearrange("p (t c) -> p t c", c=32))
            tile.add_dep_helper(d0.ins, ccs[q].ins, reason="peer q ready")
            tile.add_dep_helper(d1.ins, ccs[q].ins, reason="peer q ready")
            pm = peerp.tile([128, 32, 32], F16, tag="pm")
            nc.vector.tensor_scalar(pm[:], s1c[:, ::-1, :], sel1_sb[:, 0:1], None,
                                    op0=ALU.mult)
            pc = peerp.tile([128, 32, 32], F16, tag="pc")
            nc.vector.scalar_tensor_tensor(pc[:], s0c[:, ::-1, :], sel0_sb[:, 0:1],
                                           pm[:], op0=ALU.mult, op1=ALU.add)
            asm = gst.tile([128, 32, 128], F16, tag="asm")
            for m in range(8):
                ps = gps.tile([128, 512], F32, tag="gps")
                for k in range(4):
                    rhs = (ownc[:, :, ds(k * 16, 16)] if k < 2
                           else pc[:, :, ds((k - 2) * 16, 16)])
                    nc.tensor.matmul(ps[:], w1_sb[:, ds((k * 8 + m) * 128, 128)],
                                     rhs, start=(k == 0), stop=(k == 3))
                nc.vector.tensor_scalar(asm[:, :, ds(m * 16, 16)],
                                         ps.rearrange("p (t b) -> p t b", b=16),
                                         b1_sb[:, ds(m, 1)], None, op0=ALU.add)
            nc.sync.dma_start(xp1_d[:, ds(tci * 4096, 4096)], asm[:])

        tc.strict_bb_all_engine_barrier()
        # --- L1 scan
        _scan_range(tc, nc, st, 0, T // CH, u1_sb, xp1_d, own1_d, ident,
                    scan_pools, prologue=True)
        tc.strict_bb_all_engine_barrier()

        # --- head: y = h1_own @ wd_half (partial; host sums pairs)
        y_sb = const.tile([1, NT], F32, tag="ysb")
        for tci in range(T // 32):
            t0 = tci * 32
            own1c = peerp.tile([128, 32, 32], F16, tag="own1c")
            nc.sync.dma_start(own1c[:], own1_d[:, ds(32 * t0, 1024)]
                              .rearrange("p (t c) -> p t c", c=32))
            ps = hps.tile([1, 512], F32, tag="hps")
            for k in range(2):
                nc.tensor.matmul(ps[:], wd_sb[:, ds(k, 1)],
                                 own1c[:, :, ds(k * 16, 16)],
                                 start=(k == 0), stop=(k == 1))
            nc.vector.tensor_copy(y_sb[:, ds(tci * 512, 512)], ps[:])
        nc.sync.dma_start(y[:], y_sb[:])

    if finalize:
        nc.finalize()
    return nc


_NC_CACHE = {}


def _get_nc(T):
    if T not in _NC_CACHE:
        _NC_CACHE[T] = build_nc(T)
    return _NC_CACHE[T]


def kernel(**inputs) -> np.ndarray:
    T = np.asarray(inputs['x']).shape[1]
    in_maps = host_prep(inputs, T=T)
    nc = _get_nc(T)
    res = run_bass_kernel_spmd(nc, in_maps, core_ids=list(range(NCORES)))
    return host_post(res.results, inputs, T=T)


if __name__ == "__main__":
    rng = np.random.default_rng(0)
    Tt = 64
    demo = {
        'x': rng.standard_normal((B, Tt, F_IN), NP32),
        'Wd': rng.standard_normal((2 * H, 1), NP32) * 0.04,
        'bd': np.zeros(1, NP32),
    }
    for sfx in ('f', 'b'):
        demo[f'W0{sfx}'] = rng.standard_normal((F_IN, G4), NP32) * 0.125
        demo[f'U0{sfx}'] = rng.standard_normal((H, G4), NP32) * 0.0625
        demo[f'b0{sfx}'] = np.zeros(G4, NP32)
        demo[f'W1{sfx}'] = rng.standard_normal((2 * H, G4), NP32) * 0.044
        demo[f'U1{sfx}'] = rng.standard_normal((H, G4), NP32) * 0.0625
        demo[f'b1{sfx}'] = np.zeros(G4, NP32)
    out = kernel(**demo)
    print("kernel output:", out.shape, out.dtype, np.abs(out).max())
